# revision 5
# baseline (speedup 1.0000x reference)
"""BDH parallel attention (chunked linear attention with interleaved RoPE) on 8 TRN2 cores.

Reference computation (B=1, NH=16, T=4096, N=256, D=1024, CHUNK=128):
  QR = rope(Q); KR == QR; V head-broadcast
  per chunk c (sequential recurrence over 32 chunks, per head):
    out   = q_c @ state + (tril(q_c q_c^T, -1)) @ v_c
    state = state + q_c^T @ v_c

Sharding: head-parallel, 2 heads per core, no cross-core communication.
All matmuls run in float32r (fp32 with mantissa rounded to 11 explicit bits;
PE streams it at full rate). Operand rounding is the only numeric loss
(~1.6e-4 relative); accumulation is exact fp32 in PSUM.
"""
import math
import os
import numpy as np

B, NH, T, N, D = 1, 16, 4096, 256, 1024
C = 128                  # chunk length == partition count
NCH = T // C             # 32 chunks
HPC = NH // 8            # heads per core = 2
THETA = 2.0 ** 16
TWO_PI = 2.0 * math.pi

_CACHE = {}
LAST_EXEC_NS = None


def _round_fp32r(x: np.ndarray) -> np.ndarray:
    """fp32 -> nearest fp32r (11 explicit mantissa bits), returned as fp32 bits."""
    try:
        from neuron_dtypes import static_cast_fp32_to_fp32r
        return np.asarray(static_cast_fp32_to_fp32r(x)).view(np.float32).reshape(x.shape)
    except Exception:
        u = np.ascontiguousarray(x, dtype=np.float32).view(np.uint32)
        low = u & np.uint32(0xFFF)
        base = u & np.uint32(0xFFFFF000)
        half = np.uint32(0x800)
        round_up = (low > half) | ((low == half) & ((u >> np.uint32(12)) & np.uint32(1)).astype(bool))
        out = base + np.where(round_up, np.uint32(0x1000), np.uint32(0))
        return out.view(np.float32).reshape(x.shape)


def _tables():
    """cos/sin phase tables [T, N] in fp32, replicating the fp32 reference math."""
    t = np.floor(np.arange(N, dtype=np.float32) / np.float32(2.0)) * np.float32(2.0)
    freqs = (np.float32(1.0) / (np.float32(THETA) ** (t / np.float32(N))) / np.float32(TWO_PI)).astype(np.float32)
    pos = np.arange(T, dtype=np.float32)
    phases = pos[:, None] * freqs[None, :]
    ph = np.mod(phases, np.float32(1.0)) * np.float32(TWO_PI)
    cos_t = np.cos(ph).astype(np.float32)
    sin_t = np.sin(ph).astype(np.float32)
    # fold rot()'s sign into the table: qr_e = q_e*cos_e + q_o*(-sin_e)
    sin_signed = sin_t.copy()
    sin_signed[:, 0::2] = -sin_signed[:, 0::2]
    return cos_t, sin_signed


def _build():
    import concourse.bacc as bacc
    import concourse.mybir as mybir
    import concourse.tile as tile

    f32 = mybir.dt.float32
    f32r = mybir.dt.float32r
    P = 128

    nc = bacc.Bacc("TRN2", target_bir_lowering=False, debug=False)

    Qd = nc.dram_tensor("Q", [HPC, T, N], f32, kind="ExternalInput")
    Vd = nc.dram_tensor("V", [T, D], f32r, kind="ExternalInput")
    COSd = nc.dram_tensor("COS", [T, N], f32, kind="ExternalInput")
    SINd = nc.dram_tensor("SIN", [T, N], f32, kind="ExternalInput")
    Od = nc.dram_tensor("O", [HPC, T, D], f32, kind="ExternalOutput")

    with tile.TileContext(nc) as tc:
        with (
            tc.tile_pool(name="const", bufs=1) as constp,
            tc.tile_pool(name="vp", bufs=3) as vp,
            tc.tile_pool(name="tbl", bufs=3) as tblp,
            tc.tile_pool(name="qp", bufs=4) as qp,
            tc.tile_pool(name="ropep", bufs=4) as ropep,
            tc.tile_pool(name="qrp", bufs=4) as qrp,
            tc.tile_pool(name="qtp", bufs=4) as qtp,
            tc.tile_pool(name="stmp", bufs=4) as stmp,
            tc.tile_pool(name="ostg", bufs=6) as ostg,
            tc.tile_pool(name="st0a", bufs=2) as st_pools_00,
            tc.tile_pool(name="st0b", bufs=2) as st_pools_01,
            tc.tile_pool(name="st1a", bufs=2) as st_pools_10,
            tc.tile_pool(name="st1b", bufs=2) as st_pools_11,
            tc.tile_pool(name="dps", bufs=2, space="PSUM") as dps,
            tc.tile_pool(name="ops", bufs=2, space="PSUM") as ops,
            tc.tile_pool(name="trps", bufs=1, space="PSUM") as trps,
            tc.tile_pool(name="scps", bufs=1, space="PSUM") as scps,
        ):
            st_pools = [[st_pools_00, st_pools_01], [st_pools_10, st_pools_11]]

            # constants: identity (f32r, for PE transpose) + strict-upper mask
            ones = constp.tile([P, P], f32, tag="ones")
            ident_f = constp.tile([P, P], f32, tag="ident_f")
            identr = constp.tile([P, P], f32r, tag="identr")
            maskT = constp.tile([P, P], f32, tag="maskT")
            nc.gpsimd.memset(ones[:], 1.0)
            nc.gpsimd.affine_select(
                ident_f[:], ones[:], pattern=[[1, P]],
                compare_op=mybir.AluOpType.is_equal, fill=0.0,
                base=0, channel_multiplier=-1,
            )
            nc.vector.tensor_copy(identr[:], ident_f[:])
            # maskT[k, c] = 1 if k < c (strict upper): iota = c - k - 1 >= 0
            nc.gpsimd.affine_select(
                maskT[:], ones[:], pattern=[[1, P]],
                compare_op=mybir.AluOpType.is_ge, fill=0.0,
                base=-1, channel_multiplier=-1,
            )

            st_cur = [[None, None], [None, None]]  # [h][half] -> sbuf tile [128,1024] f32r

            for i in range(NCH):
                r0 = i * C
                v = vp.tile([P, D], f32r, tag="v")
                nc.sync.dma_start(v[:], Vd.ap()[r0:r0 + C, :])
                cos_t = tblp.tile([P, N], f32, tag="cos")
                sin_t = tblp.tile([P, N], f32, tag="sin")
                nc.sync.dma_start(cos_t[:], COSd.ap()[r0:r0 + C, :])
                nc.sync.dma_start(sin_t[:], SINd.ap()[r0:r0 + C, :])

                for h in range(HPC):
                    q = qp.tile([P, N], f32, tag="q")
                    nc.sync.dma_start(q[:], Qd.ap()[h, r0:r0 + C, :])

                    # ---- RoPE (GpSimd, SBUF only) -> qr (f32r) ----
                    # qr_e = q_e*cos_e + q_o*sin'_e ; qr_o = q_o*cos_o + q_e*sin'_o  (sin' sign-folded)
                    t1 = ropep.tile([P, N], f32, tag="t1")
                    t2 = ropep.tile([P, N], f32, tag="t2")
                    qr = qrp.tile([P, N], f32r, tag="qr")
                    q2 = q[:].rearrange("p (n two) -> p n two", two=2)
                    t12 = t1[:].rearrange("p (n two) -> p n two", two=2)
                    t22 = t2[:].rearrange("p (n two) -> p n two", two=2)
                    qr2 = qr[:].rearrange("p (n two) -> p n two", two=2)
                    s2 = sin_t[:].rearrange("p (n two) -> p n two", two=2)
                    nc.gpsimd.tensor_mul(t1[:], q[:], cos_t[:])
                    nc.gpsimd.tensor_mul(t22[:, :, 0], q2[:, :, 1], s2[:, :, 0])
                    nc.gpsimd.tensor_mul(t22[:, :, 1], q2[:, :, 0], s2[:, :, 1])
                    nc.gpsimd.tensor_add(qr[:], t2[:], t1[:])

                    # ---- transpose qr -> qT ----
                    trp = trps.tile([P, 2, P], f32, tag="trp")
                    nc.tensor.transpose(trp[:, 0, :].bitcast(f32r), qr[:, 0:P], identr[:])
                    nc.tensor.transpose(trp[:, 1, :].bitcast(f32r), qr[:, P:N], identr[:])
                    qT = qtp.tile([P, 2, P], f32r, tag="qT")
                    nc.scalar.copy(qT[:], trp[:].bitcast(f32r))

                    # scores S[c,k] = sum_n qr[c,n] qr[k,n]. rhs is the WHOLE qT
                    # tile (256 wide) so the f32r matmul streams at 1 cyc/row;
                    # columns [128:256) are don't-care cross terms never read.
                    scs = scps.tile([P, 2, P], f32, tag="scs")
                    qTflat = qT[:].rearrange("p a b -> p (a b)")
                    nc.tensor.matmul(scs[:], qT[:, 0, :], qTflat, start=True, stop=False)
                    nc.tensor.matmul(scs[:], qT[:, 1, :], qTflat, start=False, stop=True)
                    # STm[k,c] = S[k,c] * (k<c)   (S symmetric -> this is scores^T masked)
                    stm = stmp.tile([P, P], f32r, tag="stm")
                    nc.vector.tensor_tensor(stm[:], scs[:, 0, :], maskT[:], mybir.AluOpType.mult)

                    # ---- out = STm.T @ v + qT.T @ state ----
                    for dh in range(2):
                        dsl = slice(dh * 512, (dh + 1) * 512)
                        op = ops.tile([P, 512], f32, tag="op")
                        nc.tensor.matmul(op[:], stm[:], v[:, dsl],
                                         start=True, stop=(i == 0))
                        if i > 0:
                            nc.tensor.matmul(op[:], qT[:, 0, :], st_cur[h][0][:, dsl],
                                             start=False, stop=False)
                            nc.tensor.matmul(op[:], qT[:, 1, :], st_cur[h][1][:, dsl],
                                             start=False, stop=True)
                        ost = ostg.tile([P, 512], f32, tag="ost")
                        nc.scalar.copy(ost[:], op[:])
                        nc.sync.dma_start(Od.ap()[h, r0:r0 + C, dsl], ost[:])

                    # ---- state update: state[half] += qr[:, half].T @ v ----
                    if i < NCH - 1:
                        for half in range(2):
                            nsl = slice(half * P, (half + 1) * P)
                            delta = dps.tile([P, 2, 512], f32, tag="delta")
                            nc.tensor.matmul(delta[:, 0, :], qr[:, nsl], v[:, 0:512],
                                             start=True, stop=True)
                            nc.tensor.matmul(delta[:, 1, :], qr[:, nsl], v[:, 512:1024],
                                             start=True, stop=True)
                            st_new = st_pools[h][half].tile([P, D], f32r, tag=f"st{h}{half}")
                            dflat = delta[:].rearrange("p a b -> p (a b)")
                            if i == 0:
                                nc.vector.tensor_copy(st_new[:], dflat)
                            else:
                                nc.vector.tensor_tensor(
                                    st_new[:], dflat, st_cur[h][half][:],
                                    mybir.AluOpType.add,
                                )
                            st_cur[h][half] = st_new

    nc.compile()
    return nc


def _get_nc():
    if "nc" not in _CACHE:
        _CACHE["nc"] = _build()
    return _CACHE["nc"]


def kernel(**inputs) -> np.ndarray:
    global LAST_EXEC_NS
    from concourse.bass_utils import run_bass_kernel_spmd

    Q_raw = np.ascontiguousarray(np.asarray(inputs["Q_raw"], dtype=np.float32))
    V_raw = np.ascontiguousarray(np.asarray(inputs["V_raw"], dtype=np.float32))

    cos_t, sin_t = _tables()
    v_r = _round_fp32r(V_raw[0])

    nc = _get_nc()
    in_maps = []
    for c in range(8):
        in_maps.append({
            "Q": np.ascontiguousarray(Q_raw[0, c * HPC:(c + 1) * HPC]),
            "V": v_r,
            "COS": cos_t,
            "SIN": sin_t,
        })

    trace = bool(int(os.environ.get("BDH_TRACE", "0")))
    res = run_bass_kernel_spmd(nc, in_maps, core_ids=list(range(8)), trace=trace)
    LAST_EXEC_NS = res.exec_time_ns

    out = np.empty((B, NH, T, D), dtype=np.float32)
    for c in range(8):
        out[0, c * HPC:(c + 1) * HPC] = res.results[c]["O"]
    return out


# revision 6
# speedup vs baseline: 1.0558x; 1.0558x over previous
"""BDH parallel attention (chunked linear attention with interleaved RoPE) on 8 TRN2 cores.

Reference computation (B=1, NH=16, T=4096, N=256, D=1024, CHUNK=128):
  QR = rope(Q); KR == QR; V head-broadcast
  per chunk c (sequential recurrence over 32 chunks, per head):
    out   = q_c @ state + (tril(q_c q_c^T, -1)) @ v_c
    state = state + q_c^T @ v_c

Sharding: head-parallel, 2 heads per core, no cross-core communication.
All matmuls run in float32r (fp32 with mantissa rounded to 11 explicit bits;
PE streams it at full rate). Operand rounding is the only numeric loss
(~1.6e-4 relative); accumulation is exact fp32 in PSUM.
"""
import math
import os
import numpy as np

B, NH, T, N, D = 1, 16, 4096, 256, 1024
C = 128                  # chunk length == partition count
NCH = T // C             # 32 chunks
HPC = NH // 8            # heads per core = 2
THETA = 2.0 ** 16
TWO_PI = 2.0 * math.pi

_CACHE = {}
LAST_EXEC_NS = None


def _round_fp32r(x: np.ndarray) -> np.ndarray:
    """fp32 -> nearest fp32r (11 explicit mantissa bits), returned as fp32 bits."""
    try:
        from neuron_dtypes import static_cast_fp32_to_fp32r
        return np.asarray(static_cast_fp32_to_fp32r(x)).view(np.float32).reshape(x.shape)
    except Exception:
        u = np.ascontiguousarray(x, dtype=np.float32).view(np.uint32)
        low = u & np.uint32(0xFFF)
        base = u & np.uint32(0xFFFFF000)
        half = np.uint32(0x800)
        round_up = (low > half) | ((low == half) & ((u >> np.uint32(12)) & np.uint32(1)).astype(bool))
        out = base + np.where(round_up, np.uint32(0x1000), np.uint32(0))
        return out.view(np.float32).reshape(x.shape)


def _tables():
    """cos/sin phase tables [T, N] in fp32, replicating the fp32 reference math."""
    t = np.floor(np.arange(N, dtype=np.float32) / np.float32(2.0)) * np.float32(2.0)
    freqs = (np.float32(1.0) / (np.float32(THETA) ** (t / np.float32(N))) / np.float32(TWO_PI)).astype(np.float32)
    pos = np.arange(T, dtype=np.float32)
    phases = pos[:, None] * freqs[None, :]
    ph = np.mod(phases, np.float32(1.0)) * np.float32(TWO_PI)
    cos_t = np.cos(ph).astype(np.float32)
    sin_t = np.sin(ph).astype(np.float32)
    # fold rot()'s sign into the table: qr_e = q_e*cos_e + q_o*(-sin_e)
    sin_signed = sin_t.copy()
    sin_signed[:, 0::2] = -sin_signed[:, 0::2]
    return cos_t, sin_signed


def _build():
    import concourse.bacc as bacc
    import concourse.mybir as mybir
    import concourse.tile as tile

    f32 = mybir.dt.float32
    f32r = mybir.dt.float32r
    P = 128

    nc = bacc.Bacc("TRN2", target_bir_lowering=False, debug=False)

    Qd = nc.dram_tensor("Q", [HPC, T, N], f32, kind="ExternalInput")
    Vd = nc.dram_tensor("V", [T, D], f32r, kind="ExternalInput")
    CSd = nc.dram_tensor("CS", [T, 2 * N], f32, kind="ExternalInput")
    Od = nc.dram_tensor("O", [HPC, T, D], f32, kind="ExternalOutput")

    with tile.TileContext(nc) as tc:
        with (
            tc.tile_pool(name="const", bufs=1) as constp,
            tc.tile_pool(name="vp", bufs=3) as vp,
            tc.tile_pool(name="tbl", bufs=3) as tblp,
            tc.tile_pool(name="qp", bufs=4) as qp,
            tc.tile_pool(name="ropep", bufs=4) as ropep,
            tc.tile_pool(name="qrp", bufs=4) as qrp,
            tc.tile_pool(name="qtp", bufs=4) as qtp,
            tc.tile_pool(name="stmp", bufs=4) as stmp,
            tc.tile_pool(name="ostg", bufs=6) as ostg,
            tc.tile_pool(name="st0a", bufs=2) as st_pools_00,
            tc.tile_pool(name="st0b", bufs=2) as st_pools_01,
            tc.tile_pool(name="st1a", bufs=2) as st_pools_10,
            tc.tile_pool(name="st1b", bufs=2) as st_pools_11,
            tc.tile_pool(name="dps", bufs=2, space="PSUM") as dps,
            tc.tile_pool(name="ops", bufs=2, space="PSUM") as ops,
            tc.tile_pool(name="trps", bufs=1, space="PSUM") as trps,
            tc.tile_pool(name="scps", bufs=1, space="PSUM") as scps,
        ):
            st_pools = [[st_pools_00, st_pools_01], [st_pools_10, st_pools_11]]

            # constants: identity (f32r, for PE transpose) + strict-upper mask
            ones = constp.tile([P, P], f32, tag="ones")
            ident_f = constp.tile([P, P], f32, tag="ident_f")
            identr = constp.tile([P, P], f32r, tag="identr")
            maskT = constp.tile([P, P], f32, tag="maskT")
            nc.gpsimd.memset(ones[:], 1.0)
            nc.gpsimd.affine_select(
                ident_f[:], ones[:], pattern=[[1, P]],
                compare_op=mybir.AluOpType.is_equal, fill=0.0,
                base=0, channel_multiplier=-1,
            )
            nc.vector.tensor_copy(identr[:], ident_f[:])
            # maskT[k, c] = 1 if k < c (strict upper): iota = c - k - 1 >= 0
            nc.gpsimd.affine_select(
                maskT[:], ones[:], pattern=[[1, P]],
                compare_op=mybir.AluOpType.is_ge, fill=0.0,
                base=-1, channel_multiplier=-1,
            )

            st_cur = [[None, None], [None, None]]  # [h][half] -> sbuf tile [128,1024] f32r

            for i in range(NCH):
                r0 = i * C

                # ---- loads (one DMA each: v, cos|sin, q-both-heads) ----
                v = vp.tile([P, D], f32r, tag="v")
                nc.sync.dma_start(v[:], Vd.ap()[r0:r0 + C, :])
                cs = tblp.tile([P, 2, N], f32, tag="cs")   # [:,0]=cos, [:,1]=sin'
                nc.sync.dma_start(cs[:], CSd.ap()[r0:r0 + C, :].rearrange("r (a n) -> r a n", a=2))
                qq = qp.tile([P, HPC, N], f32, tag="qq")
                nc.sync.dma_start(qq[:], Qd.ap()[:, r0:r0 + C, :].rearrange("h r n -> r h n"))

                # ---- prep phase: rope -> transpose -> qT copy (ACT first in queue) ----
                qrs, qTs, stms = [], [], []
                for h in range(HPC):
                    t1 = ropep.tile([P, N], f32, tag="t1")
                    t2 = ropep.tile([P, N], f32, tag="t2")
                    qr = qrp.tile([P, N], f32r, tag="qr")
                    q2 = qq[:, h, :].rearrange("p (n two) -> p n two", two=2)
                    t22 = t2[:].rearrange("p (n two) -> p n two", two=2)
                    s2 = cs[:, 1, :].rearrange("p (n two) -> p n two", two=2)
                    nc.gpsimd.tensor_mul(t1[:], qq[:, h, :], cs[:, 0, :])
                    nc.gpsimd.tensor_mul(t22[:, :, 0], q2[:, :, 1], s2[:, :, 0])
                    nc.gpsimd.tensor_mul(t22[:, :, 1], q2[:, :, 0], s2[:, :, 1])
                    nc.gpsimd.tensor_add(qr[:], t2[:], t1[:])
                    qrs.append(qr)

                    trp = trps.tile([P, 2, P], f32, tag="trp")
                    nc.tensor.transpose(trp[:, 0, :].bitcast(f32r), qr[:, 0:P], identr[:])
                    nc.tensor.transpose(trp[:, 1, :].bitcast(f32r), qr[:, P:N], identr[:])
                    qT = qtp.tile([P, 2, P], f32r, tag="qT")
                    nc.scalar.copy(qT[:], trp[:].bitcast(f32r))
                    qTs.append(qT)

                # ---- scores + mask for both heads ----
                for h in range(HPC):
                    qT = qTs[h]
                    scs = scps.tile([P, 2, P], f32, tag="scs")
                    qTflat = qT[:].rearrange("p a b -> p (a b)")
                    nc.tensor.matmul(scs[:], qT[:, 0, :], qTflat, start=True, stop=False)
                    nc.tensor.matmul(scs[:], qT[:, 1, :], qTflat, start=False, stop=True)
                    stm = stmp.tile([P, P], f32r, tag="stm")
                    nc.vector.tensor_tensor(stm[:], scs[:, 0, :], maskT[:], mybir.AluOpType.mult)
                    stms.append(stm)

                # ---- out = STm.T @ v + qT.T @ state; evacuate + store ----
                for h in range(HPC):
                    qT, stm = qTs[h], stms[h]
                    for dh in range(2):
                        dsl = slice(dh * 512, (dh + 1) * 512)
                        op = ops.tile([P, 512], f32, tag="op")
                        nc.tensor.matmul(op[:], stm[:], v[:, dsl],
                                         start=True, stop=(i == 0))
                        if i > 0:
                            nc.tensor.matmul(op[:], qT[:, 0, :], st_cur[h][0][:, dsl],
                                             start=False, stop=False)
                            nc.tensor.matmul(op[:], qT[:, 1, :], st_cur[h][1][:, dsl],
                                             start=False, stop=True)
                        ost = ostg.tile([P, 512], f32, tag="ost")
                        nc.scalar.copy(ost[:], op[:])
                        nc.sync.dma_start(Od.ap()[h, r0:r0 + C, dsl], ost[:])

                # ---- state update: state[half] += qr[:, half].T @ v ----
                if i < NCH - 1:
                    for h in range(HPC):
                        qr = qrs[h]
                        for half in range(2):
                            nsl = slice(half * P, (half + 1) * P)
                            delta = dps.tile([P, 2, 512], f32, tag="delta")
                            nc.tensor.matmul(delta[:, 0, :], qr[:, nsl], v[:, 0:512],
                                             start=True, stop=True)
                            nc.tensor.matmul(delta[:, 1, :], qr[:, nsl], v[:, 512:1024],
                                             start=True, stop=True)
                            st_new = st_pools[h][half].tile([P, D], f32r, tag=f"st{h}{half}")
                            dflat = delta[:].rearrange("p a b -> p (a b)")
                            if i == 0:
                                nc.vector.tensor_copy(st_new[:], dflat)
                            else:
                                nc.vector.tensor_tensor(
                                    st_new[:], dflat, st_cur[h][half][:],
                                    mybir.AluOpType.add,
                                )
                            st_cur[h][half] = st_new

    nc.compile()
    return nc


def _get_nc():
    if "nc" not in _CACHE:
        _CACHE["nc"] = _build()
    return _CACHE["nc"]


def kernel(**inputs) -> np.ndarray:
    global LAST_EXEC_NS
    from concourse.bass_utils import run_bass_kernel_spmd

    Q_raw = np.ascontiguousarray(np.asarray(inputs["Q_raw"], dtype=np.float32))
    V_raw = np.ascontiguousarray(np.asarray(inputs["V_raw"], dtype=np.float32))

    cos_t, sin_t = _tables()
    cs = np.concatenate([cos_t, sin_t], axis=1)  # [T, 2N]
    v_r = _round_fp32r(V_raw[0])

    nc = _get_nc()
    in_maps = []
    for c in range(8):
        in_maps.append({
            "Q": np.ascontiguousarray(Q_raw[0, c * HPC:(c + 1) * HPC]),
            "V": v_r,
            "CS": cs,
        })

    trace = bool(int(os.environ.get("BDH_TRACE", "0")))
    res = run_bass_kernel_spmd(nc, in_maps, core_ids=list(range(8)), trace=trace)
    LAST_EXEC_NS = res.exec_time_ns

    out = np.empty((B, NH, T, D), dtype=np.float32)
    for c in range(8):
        out[0, c * HPC:(c + 1) * HPC] = res.results[c]["O"]
    return out
